# revision 20
# baseline (speedup 1.0000x reference)
"""Distributed causal attention block (QKV + RoPE + SDPA + Wo) on 8 TRN2 cores.

Sharding: tensor-parallel over heads (2 heads/core). Each core:
  phase 1: streams full x (bf16), PE-transposes tiles inline, QKV projection
           for its 2 heads + RoPE + transposes -> QT/KT/V resident in SBUF
  phase 2: causal attention per (batch, head), q-chunk-grouped PV with the
           softmax normalization folded into a P-prescale (on GpSimd)
  phase 3: AllGather attention outputs (d-sharded, in progressive t-pieces,
           tapering at the end) -> Wo e-slice -> output
Host concatenates the 8 e-slices.

The q/k columns of Wqkv (and the cos/sin tables) are permuted head-major
even/odd on the host so RoPE runs on contiguous blocks; attention scores are
invariant to a shared permutation of the head dim of Q and K.
"""
import numpy as np
import ml_dtypes
import bass_rust
import concourse.bass as bass
import concourse.mybir as mybir
from concourse.tile import TileContext
from concourse.masks import make_identity, make_causal_mask

B, L, D, H = 2, 2048, 2048, 16
HD = 128
N_CORES = 8
HPC = H // N_CORES          # heads per core = 2
ES = HPC * HD               # 256 = e-slice width per core
T = B * L                   # 4096 tokens total
TS = T // N_CORES           # 512 t per rank-block in phase 1
P = 128
SCALE = 1.0 / float(np.sqrt(HD))
NEG = -30000.0              # causal mask fill; exp(SCALE*(s+NEG)) underflows to 0
FP = mybir.dt.float32
BF = mybir.dt.bfloat16

N_TT = T // P               # 32 global t-tiles
N_LT = L // P               # 16 t-tiles per batch
N_DT = D // P               # 16 d-tiles

# attention-out AllGather pieces per batch, in units of 512-t q-chunks (4/batch)
AG_PIECES = {0: [(0, 2), (2, 4)], 1: [(0, 2), (2, 3), (3, 4)]}


def split_multi_waits(nc):
    """This walrus build allows 1 sync wait per instruction (2 for
    EventSemaphore). Tile attaches more on some instructions (tail drain,
    collective-adjacent DMAs); hoist the extras onto same-engine NoOps."""
    for f in nc.m.functions:
        for bb in f.blocks:
            new_insts = []
            changed = False
            for ins in bb.instructions:
                si = ins.sync_info
                cap = 2 if type(ins).__name__ == "InstEventSemaphore" else 1
                if si is not None and len(si.on_wait) > cap:
                    waits = list(si.on_wait)
                    for k, w in enumerate(waits[cap:]):
                        new_insts.append(mybir.InstNoOp(
                            name=f"{ins.name}-wsplit{k}", ins=[], outs=[],
                            engine=ins.engine,
                            sync_info=bass_rust.SyncInfo(on_wait=[w], on_update=[]),
                        ))
                    ins.sync_info = bass_rust.SyncInfo(
                        on_wait=waits[:cap], on_update=list(si.on_update))
                    changed = True
                new_insts.append(ins)
            if changed:
                bb.instructions.clear()
                for i2 in new_insts:
                    bb.add_instruction(i2)


def build(debug=False):
    nc = bass.Bass()
    x_c = nc.declare_dram_parameter("x_c", [T, D], BF, isOutput=False)
    wqkvT = nc.declare_dram_parameter("wqkvT", [D, 3 * ES], BF, isOutput=False)
    ce_p = nc.declare_dram_parameter("ce_p", [L, P], FP, isOutput=False)
    co_p = nc.declare_dram_parameter("co_p", [L, P], FP, isOutput=False)
    se_p = nc.declare_dram_parameter("se_p", [L, P], FP, isOutput=False)
    so_p = nc.declare_dram_parameter("so_p", [L, P], FP, isOutput=False)
    woT = nc.declare_dram_parameter("woT", [D, ES], BF, isOutput=False)
    out = nc.declare_dram_parameter("out", [ES, T], FP, isOutput=True)
    if debug:
        dbg_qt = nc.declare_dram_parameter("dbg_qt", [P, HPC * T], FP, isOutput=True)
        dbg_kt = nc.declare_dram_parameter("dbg_kt", [P, HPC * T], FP, isOutput=True)
        dbg_v = nc.declare_dram_parameter("dbg_v", [T, ES], FP, isOutput=True)
        dbg_o = nc.declare_dram_parameter("dbg_o", [ES, T], FP, isOutput=True)

    # out AllGather bounce/result per (batch, piece)
    o_bounce, ag_o = {}, {}
    for b, pieces in AG_PIECES.items():
        for (c0, c1) in pieces:
            w = (c1 - c0) * 512
            o_bounce[(b, c0)] = nc.dram_tensor(f"o_bounce{b}_{c0}", [ES, w], BF)
            ag_o[(b, c0)] = nc.dram_tensor(f"ag_o{b}_{c0}", [N_CORES * ES, w], BF,
                                           addr_space="Shared")
    rg = [list(range(N_CORES))]

    def r3(ap):  # [128, 256] -> [128, 2 heads, 2 (even/odd), 64]
        return ap.rearrange("p (h s x) -> p h s x", h=2, s=2)

    def r2(ap):  # [128, 128] -> [128, 2 heads, 64]
        return ap.rearrange("p (h x) -> p h x", h=2)

    with TileContext(nc, pool_alloc_mode="queue") as tc:
        with (
            tc.tile_pool(name="const", bufs=1) as const_pool,
            tc.tile_pool(name="resident", bufs=1) as res_pool,
            tc.tile_pool(name="wo", bufs=1) as wo_pool,
            tc.tile_pool(name="psA", bufs=2, space="PSUM") as psA,
            tc.tile_pool(name="psB", bufs=2, space="PSUM") as psB,
            tc.tile_pool(name="psC", bufs=2, space="PSUM") as psC,
            tc.tile_pool(name="psD", bufs=2, space="PSUM") as psD,
        ):
            ident = const_pool.tile([P, P], BF, name="ident")
            make_identity(nc, ident[:, :])
            cmask = const_pool.tile([P, P], FP, name="cmask")
            make_causal_mask(nc, cmask[:, :], mask_val=NEG)

            # resident through phases 1-2
            qt_sb = res_pool.tile([P, HPC * T], BF, name="qt_sb")   # [hd', h*T + t]
            kt_sb = res_pool.tile([P, HPC * T], BF, name="kt_sb")
            v_sb = res_pool.tile([P, N_TT * ES], BF, name="v_sb")   # [t%128, tt*ES+e]

            # ---------------- phase 1: x^T tiles + QKV + RoPE ----------------
            with (
                tc.tile_pool(name="wq", bufs=1) as wq_pool,
                tc.tile_pool(name="p1n", bufs=8) as p1n,
                tc.tile_pool(name="p1x", bufs=2) as p1x,
                tc.tile_pool(name="p1t", bufs=3) as p1t,
            ):
                wt_sb = wq_pool.tile([P, N_DT * 3 * ES], BF, name="wt_sb")
                trig_sb = {}
                for nm in ("ce", "co", "se", "so"):
                    trig_sb[nm] = wq_pool.tile([P, N_LT * P], FP, name=f"{nm}_sb")
                woT_sb = wo_pool.tile([P, N_DT * ES], BF, name="woT_sb")

                xins = {}

                def load_xins(rb):
                    tiles = []
                    for tl in range(TS // P):
                        xin = p1n.tile([P, D], BF, name="xin", tag="xin")
                        t0 = rb * TS + tl * P
                        nc.sync.dma_start(out=xin[:, :], in_=x_c[t0:t0 + P, :])
                        tiles.append(xin)
                    xins[rb] = tiles

                # priority: first two rank blocks of x, then weights, then trig
                load_xins(0)
                load_xins(1)
                trig_srcs = (("ce", ce_p), ("co", co_p), ("se", se_p),
                             ("so", so_p))
                for dt in range(N_DT):
                    nc.sync.dma_start(out=wt_sb[:, dt * 3 * ES:(dt + 1) * 3 * ES],
                                      in_=wqkvT[dt * P:(dt + 1) * P, :])
                    for nm, prm in trig_srcs:
                        lt = dt
                        nc.sync.dma_start(out=trig_sb[nm][:, lt * P:(lt + 1) * P],
                                          in_=prm[lt * P:(lt + 1) * P, :])

                for rb in range(N_CORES):
                    if rb + 2 < N_CORES:
                        pass  # xins loaded lazily below
                    # build x^T tiles for this rank block on the PE
                    xt_rb = p1x.tile([P, N_DT * TS], BF, name="xt_rb")
                    for dt in range(N_DT):
                        txp = psC.tile([P, TS], BF, name="txp", tag="C")
                        for tl in range(TS // P):
                            nc.tensor.transpose(
                                txp[:, tl * P:(tl + 1) * P],
                                xins[rb][tl][:, dt * P:(dt + 1) * P], ident[:, :])
                        nc.any.tensor_copy(xt_rb[:, dt * TS:(dt + 1) * TS],
                                           txp[:, :])
                    if rb + 2 < N_CORES:
                        load_xins(rb + 2)
                    if rb == N_CORES - 1:
                        for dt in range(N_DT):
                            nc.sync.dma_start(
                                out=woT_sb[:, dt * ES:(dt + 1) * ES],
                                in_=woT[dt * P:(dt + 1) * P, :])
                    for tl in range(TS // P):
                        tt = rb * (TS // P) + tl
                        lt = tt % N_LT
                        ps_qk = psA.tile([P, 2 * ES], FP, name="ps_qk", tag="A")
                        ps_v = psB.tile([P, ES], FP, name="ps_v", tag="B")
                        for dt in range(N_DT):
                            lhsT = xt_rb[:, dt * TS + tl * P: dt * TS + (tl + 1) * P]
                            nc.tensor.matmul(
                                ps_qk[:, :], lhsT,
                                wt_sb[:, dt * 3 * ES: dt * 3 * ES + 2 * ES],
                                start=(dt == 0), stop=(dt == N_DT - 1))
                            nc.tensor.matmul(
                                ps_v[:, :], lhsT,
                                wt_sb[:, dt * 3 * ES + 2 * ES:(dt + 1) * 3 * ES],
                                start=(dt == 0), stop=(dt == N_DT - 1))
                        nc.vector.tensor_copy(v_sb[:, tt * ES:(tt + 1) * ES],
                                              ps_v[:, :])
                        ce = r2(trig_sb["ce"][:, lt * P:(lt + 1) * P])
                        co = r2(trig_sb["co"][:, lt * P:(lt + 1) * P])
                        se = r2(trig_sb["se"][:, lt * P:(lt + 1) * P])
                        so = r2(trig_sb["so"][:, lt * P:(lt + 1) * P])
                        for part in range(2):  # 0=q, 1=k
                            src = r3(ps_qk[:, part * ES:(part + 1) * ES])
                            e_, o_ = src[:, :, 0, :], src[:, :, 1, :]
                            rot = p1t.tile([P, ES], BF, name="rot", tag="rot")
                            rdst = r3(rot[:, :])
                            re_, ro_ = rdst[:, :, 0, :], rdst[:, :, 1, :]
                            t1 = p1t.tile([P, P], FP, name="t1", tag="t1")
                            t2 = p1t.tile([P, P], FP, name="t2", tag="t2")
                            t13, t23 = r2(t1[:, :]), r2(t2[:, :])
                            nc.vector.tensor_tensor(t13, e_, ce,
                                                    op=mybir.AluOpType.mult)
                            nc.vector.tensor_tensor(t23, o_, se,
                                                    op=mybir.AluOpType.mult)
                            nc.vector.tensor_tensor(re_, t13, t23,
                                                    op=mybir.AluOpType.subtract)
                            nc.vector.tensor_tensor(t13, o_, co,
                                                    op=mybir.AluOpType.mult)
                            nc.vector.tensor_tensor(t23, e_, so,
                                                    op=mybir.AluOpType.mult)
                            nc.vector.tensor_tensor(ro_, t13, t23,
                                                    op=mybir.AluOpType.add)
                            dst = qt_sb if part == 0 else kt_sb
                            for h in range(HPC):
                                tps = psD.tile([P, P], BF, name="tps", tag="D")
                                nc.tensor.transpose(
                                    tps[:, :], rot[:, h * HD:(h + 1) * HD],
                                    ident[:, :])
                                nc.vector.tensor_copy(
                                    dst[:, h * T + tt * P: h * T + (tt + 1) * P],
                                    tps[:, :])

            if debug:
                with tc.tile_pool(name="dbgp", bufs=2) as dbgp:
                    for nm, src in (("dbg_qt", qt_sb), ("dbg_kt", kt_sb)):
                        dd = {"dbg_qt": dbg_qt, "dbg_kt": dbg_kt}[nm]
                        for i in range(HPC * T // 512):
                            s = dbgp.tile([P, 512], FP, name="dstage")
                            nc.vector.tensor_copy(s[:, :],
                                                  src[:, i * 512:(i + 1) * 512])
                            nc.sync.dma_start(out=dd[:, i * 512:(i + 1) * 512],
                                              in_=s[:, :])
                    for tt in range(N_TT):
                        s = dbgp.tile([P, ES], FP, name="dstage2")
                        nc.vector.tensor_copy(s[:, :], v_sb[:, tt * ES:(tt + 1) * ES])
                        nc.sync.dma_start(out=dbg_v[tt * P:(tt + 1) * P, :],
                                          in_=s[:, :])

            # ---------------- phases 2+3 (shared pools, interleaved) ----------
            with (
                tc.tile_pool(name="p2p", bufs=8) as p2p,
                tc.tile_pool(name="p2pt", bufs=4) as p2pt,
                tc.tile_pool(name="p2sm", bufs=4) as p2sm,
                tc.tile_pool(name="p2ob", bufs=2) as p2ob,
                tc.tile_pool(name="p3x", bufs=2) as p3x,
                tc.tile_pool(name="p3o", bufs=2) as p3o,
            ):
                def phase2(b):
                    ob_sb = p2ob.tile([P, HPC * L], BF, name="ob_sb", tag="ob")
                    for qc in range(4):
                        for h in range(HPC):
                            qoff = h * T + b * L
                            psbs = []
                            for qi in range(qc * 4, qc * 4 + 4):
                                kend = (qi + 1) * P
                                nch = (kend + 511) // 512
                                p_sb = p2p.tile([P, L], BF, name="p_sb", tag="p")
                                sums = p2sm.tile([P, 4], FP, name="sums", tag="sums")
                                for ci in range(nch):
                                    klo = ci * 512
                                    ksz = min(512, kend - klo)
                                    s_ps = psA.tile([P, 512], FP, name="s_ps", tag="A")
                                    nc.tensor.matmul(
                                        s_ps[:, :ksz],
                                        qt_sb[:, qoff + qi * P: qoff + (qi + 1) * P],
                                        kt_sb[:, qoff + klo: qoff + klo + ksz],
                                        start=True, stop=True)
                                    if klo + ksz == kend:  # diagonal 128-block
                                        dslice = s_ps[:, ksz - P:ksz]
                                        nc.vector.tensor_tensor(
                                            dslice, dslice, cmask[:, :],
                                            op=mybir.AluOpType.add)
                                    nc.scalar.activation(
                                        p_sb[:, klo:klo + ksz], s_ps[:, :ksz],
                                        mybir.ActivationFunctionType.Exp,
                                        scale=SCALE,
                                        accum_out=sums[:, ci:ci + 1])
                                tot = p2sm.tile([P, 1], FP, name="tot", tag="tot")
                                if nch > 1:
                                    nc.vector.tensor_reduce(
                                        tot[:, :], sums[:, :nch],
                                        axis=mybir.AxisListType.X,
                                        op=mybir.AluOpType.add)
                                else:
                                    nc.vector.tensor_copy(tot[:, :], sums[:, 0:1])
                                rec = p2sm.tile([P, 1], FP, name="rec", tag="rec")
                                nc.vector.reciprocal(rec[:, :], tot[:, :])
                                nc.vector.tensor_scalar_mul(
                                    p_sb[:, :kend], p_sb[:, :kend], rec[:, 0:1])
                                psbs.append(p_sb)
                            # PV for this q-chunk: o^T [hd, 512 q]
                            o_ps = psB.tile([P, 512], FP, name="o_ps", tag="B")
                            for kt in range(qc * 4 + 4):
                                off = max(0, kt * P - qc * 512)
                                pt_ps = psC.tile([P, 512], BF, name="pt_ps", tag="C")
                                for j in range(4):
                                    qi = qc * 4 + j
                                    if kt <= qi:
                                        nc.tensor.transpose(
                                            pt_ps[:, j * P:(j + 1) * P],
                                            psbs[j][:, kt * P:(kt + 1) * P],
                                            ident[:, :])
                                pt_sb = p2pt.tile([P, 512], BF, name="pt_sb")
                                nc.vector.tensor_copy(pt_sb[:, off:], pt_ps[:, off:])
                                nc.tensor.matmul(
                                    o_ps[:, off:],
                                    v_sb[:, (b * N_LT + kt) * ES + h * HD:
                                         (b * N_LT + kt) * ES + (h + 1) * HD],
                                    pt_sb[:, off:],
                                    start=(kt == 0), stop=(kt == qc * 4 + 3))
                            nc.vector.tensor_copy(
                                ob_sb[:, h * L + qc * 512:h * L + (qc + 1) * 512],
                                o_ps[:, :])
                        # piece boundary: bounce + AllGather
                        for (c0, c1) in AG_PIECES[b]:
                            if c1 == qc + 1:
                                for h in range(HPC):
                                    nc.sync.dma_start(
                                        out=o_bounce[(b, c0)][h * HD:(h + 1) * HD, :],
                                        in_=ob_sb[:, h * L + c0 * 512:
                                                  h * L + c1 * 512])
                                nc.gpsimd.collective_compute(
                                    "AllGather", mybir.AluOpType.bypass,
                                    ins=[o_bounce[(b, c0)][:]],
                                    outs=[ag_o[(b, c0)][:]],
                                    replica_groups=rg)

                def phase3(b, c0, c1):
                    w = (c1 - c0) * 512
                    for tch in range(w // 512):
                        ot_ch = p3x.tile([P, N_DT * 512], BF, name="ot_ch")
                        for dt in range(N_DT):
                            nc.sync.dma_start(
                                out=ot_ch[:, dt * 512:(dt + 1) * 512],
                                in_=ag_o[(b, c0)][dt * P:(dt + 1) * P,
                                                  tch * 512:(tch + 1) * 512])
                        t0 = b * L + (c0 + tch) * 512
                        for et in range(2):
                            f_ps = psD.tile([P, 512], FP, name="f_ps", tag="D")
                            for dt in range(N_DT):
                                nc.tensor.matmul(
                                    f_ps[:, :],
                                    woT_sb[:, dt * ES + et * P:
                                           dt * ES + (et + 1) * P],
                                    ot_ch[:, dt * 512:(dt + 1) * 512],
                                    start=(dt == 0), stop=(dt == N_DT - 1))
                            f_sb = p3o.tile([P, 512], FP, name="f_sb")
                            nc.vector.tensor_copy(f_sb[:, :], f_ps[:, :])
                            nc.sync.dma_start(
                                out=out[et * P:(et + 1) * P, t0:t0 + 512],
                                in_=f_sb[:, :])

                phase2(0)
                phase2(1)
                for b, pieces in AG_PIECES.items():
                    for (c0, c1) in pieces:
                        phase3(b, c0, c1)

            if debug:
                with tc.tile_pool(name="dbgo", bufs=2) as dbgo:
                    for b, pieces in AG_PIECES.items():
                        for (c0, c1) in pieces:
                            w = (c1 - c0) * 512
                            for i in range(HPC):
                                s = dbgo.tile([P, 2048], FP, name="dob")
                                stg = dbgo.tile([P, 2048], BF, name="dob_b")
                                nc.sync.dma_start(
                                    out=stg[:, :w],
                                    in_=o_bounce[(b, c0)][i * HD:(i + 1) * HD, :])
                                nc.vector.tensor_copy(s[:, :w], stg[:, :w])
                                nc.sync.dma_start(
                                    out=dbg_o[i * HD:(i + 1) * HD,
                                              b * L + c0 * 512:
                                              b * L + c0 * 512 + w],
                                    in_=s[:, :w])

    split_multi_waits(nc)
    return nc


def make_in_maps(x, cos, sin, Wqkv, Wo):
    bf = ml_dtypes.bfloat16
    xf = np.ascontiguousarray(np.asarray(x).reshape(T, D)).astype(bf)
    # q/k column permutation: head-major, evens then odds
    perm = []
    for h in range(HPC):
        perm.extend(h * HD + 2 * np.arange(64))
        perm.extend(h * HD + 2 * np.arange(64) + 1)
    perm = np.asarray(perm)
    epick = np.concatenate([h * HD + 2 * np.arange(64) for h in range(HPC)])
    in_maps = []
    for c in range(N_CORES):
        cols = slice(c * ES, (c + 1) * ES)
        wq = Wqkv[c * ES:(c + 1) * ES, :][perm]
        wk = Wqkv[D + c * ES: D + (c + 1) * ES, :][perm]
        wv = Wqkv[2 * D + c * ES: 2 * D + (c + 1) * ES, :]
        w_c = np.concatenate([wq, wk, wv], axis=0)
        cos_c = np.asarray(cos)[:, cols]
        sin_c = np.asarray(sin)[:, cols]
        in_maps.append({
            "x_c": xf,
            "wqkvT": np.ascontiguousarray(w_c.T.astype(bf)),
            "ce_p": np.ascontiguousarray(cos_c[:, epick]).astype(np.float32),
            "co_p": np.ascontiguousarray(cos_c[:, epick + 1]).astype(np.float32),
            "se_p": np.ascontiguousarray(sin_c[:, epick]).astype(np.float32),
            "so_p": np.ascontiguousarray(sin_c[:, epick + 1]).astype(np.float32),
            "woT": np.ascontiguousarray(Wo[cols, :].T.astype(bf)),
        })
    return in_maps


_cache = {}


def kernel(x, cos, sin, Wqkv, Wo):
    from concourse.bass_utils import run_bass_kernel_spmd
    x = np.asarray(x, dtype=np.float32)
    cos = np.asarray(cos, dtype=np.float32)
    sin = np.asarray(sin, dtype=np.float32)
    Wqkv = np.asarray(Wqkv, dtype=np.float32)
    Wo = np.asarray(Wo, dtype=np.float32)
    if "nc" not in _cache:
        _cache["nc"] = build()
    nc = _cache["nc"]
    in_maps = make_in_maps(x, cos, sin, Wqkv, Wo)
    res = run_bass_kernel_spmd(nc, in_maps, core_ids=list(range(N_CORES)))
    pieces = [res.results[c]["out"].T for c in range(N_CORES)]
    return np.concatenate(pieces, axis=1).reshape(B, L, D)
